# revision 2
# baseline (speedup 1.0000x reference)
"""Trainium2 Bass kernel for nn_BlockBucket (3x eres_block + basic_block).

Strategy: the per-pixel dynamic conv (filters from a 72-entry embedding table
indexed by `buckets`) is computed as bucket-sorted matmuls: pixels are grouped
by bucket into 64-pixel tiles (host-side index prep only -- all FLOPs on
device), patches are built with one dma_gather per block from an AllGathered
pixel-major conv1 output, and each tile does 9 accumulating K=64 matmuls
against its bucket's filter.  Everything between the 3x3 convs is pointwise and
stays in the bucket-sorted "slot" domain.  2 AllGathers/block (conv1out image,
o_k slots), none after block 3 (host reassembles from per-core slot outputs).
"""

import sys

sys.path.insert(0, "/opt/trn_rl_repo")

import numpy as np
import ml_dtypes

BF16 = ml_dtypes.bfloat16

# problem constants
C = 64
H = W = 64
NPIX = H * W            # 4096
NTYPES = 72
KK = 9                  # 3x3
EMB_DIM = C * (C * KK + 1)
GROUP = 4
NCORES = 8

# layout constants
TS = 64                 # slot tile size
S_MAX = 12              # tiles per core (supports up to 96 tiles globally)
S = S_MAX * TS          # 768 slots per core
S_PAD = S + 64          # +64 guaranteed-zero rows in the AG2 contribution
ZERO_SLOT = S           # rank-0 row index of a zero row (global row = S)
ROWS_PER_CORE = H // NCORES     # 8 image rows per strip
PIX_PER_CORE = ROWS_PER_CORE * W  # 512
PW = 66                 # padded row width for strip layout
STRIP_R = 12            # strip tile rows: 8 interior + 2 halo + 2 margin
IG_N = 896              # image-gather num_idxs (>= STRIP_R*PW=792, %128==0)
PG_N = KK * S           # patch-gather num_idxs = 6912 (%128==0)


def _reflect(v, n=64):
    if v < 0:
        return -v
    if v >= n:
        return 2 * n - 2 - v
    return v


def _wrap_idx(idx, n):
    """int16 index array -> [128, n//16] layout (j -> [j%16, j//16])."""
    assert len(idx) == n and n % 16 == 0
    blk = np.asarray(idx, np.int16).reshape(n // 16, 16).T
    return np.tile(blk, (8, 1))


def _host_prep(x, buckets, params):
    """Build per-core input maps + assembly info. All numpy."""
    x = np.asarray(x, np.float32).reshape(C, NPIX)
    bk = np.asarray(buckets, np.int64).reshape(NPIX)

    # ---- slot assignment (shared by all 3 blocks) ----
    tiles = []  # list of (bucket, [pixels padded with -1 to TS])
    for t in range(NTYPES):
        pix = np.nonzero(bk == t)[0]
        for off in range(0, max(len(pix), 1), TS):
            chunk = pix[off : off + TS]
            if len(chunk) == 0:
                continue
            pad = np.full(TS, -1, np.int64)
            pad[: len(chunk)] = chunk
            tiles.append((t, pad))
    n_tiles = len(tiles)
    assert n_tiles <= NCORES * S_MAX, f"too many tiles: {n_tiles}"
    # round-robin so per-core counts balance
    core_tiles = [[] for _ in range(NCORES)]
    for i, tl in enumerate(tiles):
        core_tiles[i % NCORES].append(tl)
    for i in range(NCORES):
        while len(core_tiles[i]) < S_MAX:
            core_tiles[i].append((0, np.full(TS, -1, np.int64)))

    slot2pix = np.full((NCORES, S), -1, np.int64)     # core, slot -> pixel
    tile_bucket = np.zeros((NCORES, S_MAX), np.int64)
    for i in range(NCORES):
        for t, (b, pads) in enumerate(core_tiles[i]):
            tile_bucket[i, t] = b
            slot2pix[i, t * TS : (t + 1) * TS] = pads
    pix2gslot = np.full(NPIX, -1, np.int64)           # pixel -> global AG2 row
    for i in range(NCORES):
        for s in range(S):
            p = slot2pix[i, s]
            if p >= 0:
                pix2gslot[p] = i * S_PAD + s
    assert (pix2gslot >= 0).all()

    # ---- index arrays ----
    pgidx = np.zeros((NCORES, PG_N), np.int64)        # patch gather
    for i in range(NCORES):
        for j in range(KK):
            dy, dx = j // 3 - 1, j % 3 - 1
            for s in range(S):
                p = slot2pix[i, s]
                if p < 0:
                    v = 0
                else:
                    y, xx = divmod(int(p), W)
                    v = _reflect(y + dy) * W + _reflect(xx + dx)
                pgidx[i, j * S + s] = v

    igidx = np.full((NCORES, IG_N), ZERO_SLOT, np.int64)  # image gather
    for i in range(NCORES):
        base = 8 * i * PW
        for j in range(STRIP_R * PW):
            r, cc = divmod(base + j, PW)
            y, xx = r - 1, cc - 1
            if 0 <= y < H and 0 <= xx < W:
                igidx[i, j] = pix2gslot[y * W + xx]

    xsidx = np.maximum(slot2pix, 0)                   # x0-at-slots gather

    # ---- weights ----
    def embw(emb):
        e = np.asarray(emb, np.float32).reshape(NTYPES, C, C * KK + 1)
        wf = e[:, :, : C * KK].reshape(NTYPES, C, C, KK)  # [t, o, c, kk]
        bias = e[:, :, -1]                                # [t, o]
        return wf, bias

    def conv1_bd(w1):
        # grouped (64,16,3,3) -> block-diag lhsT chunks [ci, kk, o]
        w1 = np.asarray(w1, np.float32)
        out = np.zeros((C, KK, C), np.float32)
        gs = C // GROUP
        for o in range(C):
            g = o // gs
            for cl in range(gs):
                out[g * gs + cl, :, o] = w1[o, cl].reshape(KK)
        return out

    repl = {}
    repl["ident"] = np.eye(128, dtype=np.float32).astype(BF16)
    x0pm = np.zeros((NPIX, 128), np.float32)
    x0pm[:, :C] = x.T
    repl["x0pm"] = x0pm.astype(BF16)
    for k, pre in ((1, "b1"), (2, "b2"), (3, "b3")):
        repl[f"w1bd{k}"] = conv1_bd(params[pre + "_w1"]).astype(BF16)
        repl[f"b1_{k}"] = np.asarray(params[pre + "_b1"], np.float32).reshape(C, 1)
        # w2 (64,64,1,1) -> lhsT [c, o]
        repl[f"w2t{k}"] = (
            np.asarray(params[pre + "_w2"], np.float32).reshape(C, C).T.copy()
        ).astype(BF16)
        repl[f"b2_{k}"] = np.asarray(params[pre + "_b2"], np.float32).reshape(C, 1)
    for k, cn in ((1, "c1"), (2, "c2"), (3, "c3")):
        cw = np.asarray(params[cn + "_w"], np.float32).reshape(C, C * (k + 1))
        # chunks [ci, j, o]
        repl[f"cw{k}"] = (
            cw.reshape(C, k + 1, C).transpose(2, 1, 0).copy()
        ).astype(BF16)
        repl[f"cb{k}"] = np.asarray(params[cn + "_b"], np.float32).reshape(C, 1)

    in_maps = []
    for i in range(NCORES):
        m = dict(repl)
        # x strip [64, 12, 66] zero-padded
        xs = np.zeros((C, STRIP_R, PW), np.float32)
        base = 8 * i * PW
        for j in range(STRIP_R * PW):
            r, cc = divmod(base + j, PW)
            y, xx = r - 1, cc - 1
            if 0 <= y < H and 0 <= xx < W:
                xs[:, j // PW, j % PW] = x[:, y * W + xx]
        m["xstrip"] = xs.astype(BF16)
        m["pgidx"] = _wrap_idx(pgidx[i], PG_N)
        m["igidx"] = _wrap_idx(igidx[i], IG_N)
        m["xsidx"] = _wrap_idx(xsidx[i], S)
        for k in (1, 2, 3):
            emb = params[f"b{k}_emb"]
            wf, bias = embw(emb)
            wloc = np.zeros((C, S_MAX, KK, C), np.float32)  # [ci, t, kk, o]
            bloc = np.zeros((C, S_MAX), np.float32)
            for t in range(S_MAX):
                b = tile_bucket[i, t]
                wloc[:, t, :, :] = wf[b].transpose(1, 2, 0)  # [c, kk, o]
                bloc[:, t] = bias[b]
            m[f"wloc{k}"] = wloc.astype(BF16)
            m[f"bloc{k}"] = bloc
        in_maps.append(m)

    return in_maps, slot2pix


def _emulate_core(m, k_blocks=3):
    """Pure-numpy mirror of the device program for ONE core, given its
    in_map plus the AllGather results (computed by _emulate below)."""
    raise NotImplementedError  # see _emulate


def _emulate(in_maps):
    """Numpy emulation of the full 8-core device program (validates all
    index/layout logic; mirrors device ops incl. bf16 rounding points)."""
    f32 = np.float32

    def bf(a):
        return a.astype(BF16)

    x0s = []
    for i in range(NCORES):
        g = _gather_np(in_maps[i]["x0pm"], in_maps[i]["xsidx"], S)
        x0s.append(g[:C].astype(f32))

    strip = [np.asarray(in_maps[i]["xstrip"], f32) for i in range(NCORES)]
    bslots = [[] for _ in range(NCORES)]
    oslot_prev = [None] * NCORES
    out = [None] * NCORES

    for k in (1, 2, 3):
        # conv1 on strips + relu
        ag1 = np.zeros((NPIX, 128), f32)
        for i in range(NCORES):
            w1 = np.asarray(in_maps[i][f"w1bd{k}"], f32)
            b1 = in_maps[i][f"b1_{k}"]
            ps = np.zeros((C, 8, 64), f32)
            for j in range(KK):
                dy, dx = j // 3, j % 3
                rhs = strip[i][:, dy : dy + 8, dx : dx + 64]
                ps += np.einsum("co,crw->orw", w1[:, j, :], rhs)
            c1 = np.maximum(ps.reshape(C, PIX_PER_CORE) + b1, 0)
            ag1[i * PIX_PER_CORE : (i + 1) * PIX_PER_CORE, :C] = bf(c1).T
        ag1 = bf(ag1)

        ag2 = np.zeros((NCORES * S_PAD, 128), f32)
        for i in range(NCORES):
            m = in_maps[i]
            patches = _gather_np(ag1, m["pgidx"], PG_N).astype(f32)  # [128, PG_N]
            wloc = np.asarray(m[f"wloc{k}"], f32)
            bloc = m[f"bloc{k}"]
            lrelu = np.zeros((C, S), f32)
            for t in range(S_MAX):
                acc = np.zeros((C, TS), f32)
                for j in range(KK):
                    rhs = patches[:C, j * S + t * TS : j * S + (t + 1) * TS]
                    acc += wloc[:, t, j, :].T @ rhs
                lrelu[:, t * TS : (t + 1) * TS] = np.maximum(
                    acc + bloc[:, t : t + 1], 0
                )
            lrelu = bf(lrelu).astype(f32)
            xs = x0s[i] if k == 1 else oslot_prev[i]
            w2t = np.asarray(m[f"w2t{k}"], f32)
            ps2 = w2t.T @ lrelu + xs  # identity-matmul residual
            bslot = np.maximum(ps2 + m[f"b2_{k}"], 0)
            bslot = bf(bslot).astype(f32)
            bslots[i].append(bslot)
            chain = [x0s[i]] + bslots[i]
            cwk = np.asarray(m[f"cw{k}"], f32)
            ps3 = np.zeros((C, S), f32)
            for j, rt in enumerate(chain):
                ps3 += cwk[:, j, :].T @ bf(rt.astype(f32)).astype(f32)
            ok = np.maximum(ps3 + m[f"cb{k}"], 0)
            if k == 3:
                out[i] = ok.astype(f32)
            else:
                okb = bf(ok)
                ag2[i * S_PAD : i * S_PAD + S, :C] = okb.T
                oslot_prev[i] = okb.astype(f32)
        if k < 3:
            ag2 = bf(ag2)
            for i in range(NCORES):
                g = _gather_np(ag2, in_maps[i]["igidx"], IG_N)
                strip[i] = (
                    g[:C, : STRIP_R * PW].reshape(C, STRIP_R, PW).astype(f32)
                )
    return out


def _gather_np(src, widx, n):
    """numpy mirror of dma_gather(transpose=True, elem=128): out[128, n]."""
    idx = widx[:16].T.reshape(-1)[:n].astype(np.int64)
    return np.asarray(src, np.float32)[idx].T.copy()  # [128, n]


def _assemble(outs, slot2pix):
    img = np.zeros((C, NPIX), np.float32)
    for i in range(NCORES):
        o = np.asarray(outs[i], np.float32)
        sel = slot2pix[i] >= 0
        img[:, slot2pix[i][sel]] = o[:, np.nonzero(sel)[0]]
    return img.reshape(1, C, H, W)


# ---------------------------------------------------------------------------
# bass graph
# ---------------------------------------------------------------------------


def _build_nc(trunc=0):
    import os
    import concourse.bass as bass
    import concourse.bacc as bacc
    import concourse.mybir as mybir
    import concourse.tile as tile

    dt = mybir.dt
    AF = mybir.ActivationFunctionType
    RG = [list(range(NCORES))]

    nc = bacc.Bacc(
        "TRN2",
        target_bir_lowering=False,
        debug=False,
        num_devices=NCORES,
    )

    # ---- parameters ----
    P = {}

    def param(name, shape, dtype):
        P[name] = nc.declare_dram_parameter(name, list(shape), dtype, False)

    param("xstrip", (C, STRIP_R, PW), dt.bfloat16)
    param("x0pm", (NPIX, 128), dt.bfloat16)
    param("ident", (128, 128), dt.bfloat16)
    param("pgidx", (128, PG_N // 16), dt.int16)
    param("igidx", (128, IG_N // 16), dt.int16)
    param("xsidx", (128, S // 16), dt.int16)
    for k in (1, 2, 3):
        param(f"w1bd{k}", (C, KK, C), dt.bfloat16)
        param(f"b1_{k}", (C, 1), dt.float32)
        param(f"w2t{k}", (C, C), dt.bfloat16)
        param(f"b2_{k}", (C, 1), dt.float32)
        param(f"cw{k}", (C, k + 1, C), dt.bfloat16)
        param(f"cb{k}", (C, 1), dt.float32)
        param(f"wloc{k}", (C, S_MAX, KK, C), dt.bfloat16)
        param(f"bloc{k}", (C, S_MAX), dt.float32)
    out_p = nc.declare_dram_parameter("out", [C, S], dt.float32, True)

    with tile.TileContext(nc) as tc:
        with (
            tc.tile_pool(name="wpool", bufs=1) as wpool,
            tc.tile_pool(name="work", bufs=1) as work,
            tc.tile_pool(name="psA", bufs=2, space=bass.MemorySpace.PSUM) as psA,
            tc.tile_pool(name="psB", bufs=1, space=bass.MemorySpace.PSUM) as psB,
            tc.tile_pool(name="dram", bufs=1, space=bass.MemorySpace.DRAM) as dram,
        ):
            # ---- load replicated/static data into SBUF ----
            sb = {}
            for name in P:
                if name == "x0pm":
                    continue  # DRAM gather source, not SBUF-resident
                t = wpool.tile(list(P[name].shape), P[name].dtype, name=f"sb_{name}")
                nc.sync.dma_start(t[:], P[name].ap())
                sb[name] = t

            ident = sb["ident"]

            # zero tile for AG2 zero rows
            zt = work.tile([64, 128], dt.bfloat16, name="zt")
            nc.vector.memset(zt[:], 0.0)

            # x0 at slots
            x0g = work.tile([128, 1, S], dt.bfloat16, name="x0g")
            nc.gpsimd.dma_gather(
                x0g[:], P["x0pm"].ap(), sb["xsidx"][:], S, S, 128, transpose=True
            )
            x0s = x0g[0:64, 0, :]

            strip_in = sb["xstrip"][:, :, :]
            bslot = {}
            chain = [x0s]

            for k in (1, 2, 3):
                # ---------- conv1 (strip) ----------
                ps1 = psB.tile([C, 8, 64], dt.float32, name=f"ps1_{k}", tag="ps1")
                for j in range(KK):
                    dy, dx = j // 3, j % 3
                    nc.tensor.matmul(
                        ps1[:],
                        sb[f"w1bd{k}"][:, j, :],
                        strip_in[:, dy : dy + 8, dx : dx + 64],
                        start=(j == 0),
                        stop=(j == KK - 1),
                    )
                c1 = work.tile([C, PIX_PER_CORE], dt.bfloat16, name=f"c1_{k}")
                nc.scalar.activation(
                    c1[:],
                    ps1.rearrange("p a b -> p (a b)"),
                    AF.Relu,
                    bias=sb[f"b1_{k}"][:],
                )

                # ---------- transpose to pixel-major + AG1 ----------
                pm1 = work.tile([128, 4, 128], dt.bfloat16, name=f"pm1_{k}")
                nc.vector.memset(pm1[:], 0.0)
                for cc in range(4):
                    pst = psA.tile([128, 64], dt.bfloat16, name=f"pst1_{k}_{cc}", tag="pst")
                    nc.tensor.transpose(
                        pst[:], c1[:, 128 * cc : 128 * (cc + 1)], ident[0:64, 0:64]
                    )
                    nc.scalar.activation(pm1[:, cc, 0:64], pst[:], AF.Copy)
                ag1in = dram.tile([PIX_PER_CORE, 128], dt.bfloat16, name=f"ag1in_{k}")
                nc.sync.dma_start(
                    ag1in.rearrange("(c r) e -> r c e", r=128), pm1[:]
                )
                c1pm = dram.tile(
                    [NPIX, 128], dt.bfloat16, name=f"c1pm_{k}", addr_space="Shared"
                )
                nc.gpsimd.collective_compute(
                    "AllGather",
                    mybir.AluOpType.bypass,
                    replica_groups=RG,
                    ins=[ag1in[:].opt()],
                    outs=[c1pm[:].opt()],
                )

                # ---------- patch gather ----------
                patches = work.tile([128, 1, PG_N], dt.bfloat16, name=f"patches_{k}")
                PGC = PG_N // 9  # noqa trunc-anchor
                for g3 in range(9):
                    nc.gpsimd.dma_gather(
                        patches[:, :, PGC * g3 : PGC * (g3 + 1)],
                        c1pm[:],
                        sb["pgidx"][:, (PGC // 16) * g3 : (PGC // 16) * (g3 + 1)],
                        PGC,
                        PGC,
                        128,
                        transpose=True,
                    )
                if trunc == 1:
                    tr = work.tile([C, S], dt.float32, name="trout")
                    nc.scalar.activation(tr[:], patches[0:64, 0, 0:S], AF.Copy)
                    nc.sync.dma_start(out_p.ap(), tr[:])
                    break

                # ---------- local conv per tile ----------
                lrelu = work.tile([C, S], dt.bfloat16, name=f"lrelu_{k}")
                for t in range(S_MAX):
                    psl = psA.tile([C, TS], dt.float32, name=f"psl_{k}_{t}", tag="psl")
                    for j in range(KK):
                        nc.tensor.matmul(
                            psl[:],
                            sb[f"wloc{k}"][:, t, j, :],
                            patches[0:64, 0, j * S + t * TS : j * S + (t + 1) * TS],
                            start=(j == 0),
                            stop=(j == KK - 1),
                        )
                    nc.scalar.activation(
                        lrelu[:, t * TS : (t + 1) * TS],
                        psl[:],
                        AF.Relu,
                        bias=sb[f"bloc{k}"][:, t : t + 1],
                    )

                if trunc == 2:
                    tr = work.tile([C, S], dt.float32, name="trout")
                    nc.scalar.activation(tr[:], lrelu[:], AF.Copy)
                    nc.sync.dma_start(out_p.ap(), tr[:])
                    break

                # ---------- conv2 + residual ----------
                xs = chain[-1] if k > 1 else x0s  # o_{k-1} slots (or x0)
                bs = work.tile([C, S], dt.bfloat16, name=f"bslot_{k}")
                for ch0 in range(0, S, 384):
                    sl = slice(ch0, ch0 + 384)
                    ps2 = psA.tile([C, 384], dt.float32, name=f"ps2_{k}_{ch0}", tag="ps2")
                    nc.tensor.matmul(
                        ps2[:], sb[f"w2t{k}"][:], lrelu[:, sl], start=True, stop=False
                    )
                    nc.tensor.matmul(
                        ps2[:], ident[0:64, 0:64], xs[:, sl], start=False, stop=True
                    )
                    nc.scalar.activation(
                        bs[:, sl], ps2[:], AF.Relu, bias=sb[f"b2_{k}"][:]
                    )
                bslot[k] = bs
                chain_k = [x0s] + [bslot[j][:, :] for j in range(1, k + 1)]

                # ---------- basic block ----------
                odt = dt.float32 if k == 3 else dt.bfloat16
                ok = work.tile([C, S], odt, name=f"oslot_{k}")
                for ch0 in range(0, S, 384):
                    sl = slice(ch0, ch0 + 384)
                    ps3 = psA.tile([C, 384], dt.float32, name=f"ps3_{k}_{ch0}", tag="ps2")
                    for j, rt in enumerate(chain_k):
                        nc.tensor.matmul(
                            ps3[:],
                            sb[f"cw{k}"][:, j, :],
                            rt[:, sl],
                            start=(j == 0),
                            stop=(j == len(chain_k) - 1),
                        )
                    nc.scalar.activation(
                        ok[:, sl], ps3[:], AF.Relu, bias=sb[f"cb{k}"][:]
                    )

                if k == 3 or trunc == 3:
                    if trunc == 3 and k != 3:
                        tr = work.tile([C, S], dt.float32, name="trout")
                        nc.scalar.activation(tr[:], ok[:], AF.Copy)
                        nc.sync.dma_start(out_p.ap(), tr[:])
                    else:
                        nc.sync.dma_start(out_p.ap(), ok[:])
                    break

                chain.append(ok[:, :])

                # ---------- AG2 + image gather for next block ----------
                pm2 = work.tile([128, 6, 128], dt.bfloat16, name=f"pm2_{k}")
                nc.vector.memset(pm2[:], 0.0)
                for cc in range(6):
                    pst2 = psA.tile([128, 64], dt.bfloat16, name=f"pst2_{k}_{cc}", tag="pst")
                    nc.tensor.transpose(
                        pst2[:], ok[:, 128 * cc : 128 * (cc + 1)], ident[0:64, 0:64]
                    )
                    nc.scalar.activation(pm2[:, cc, 0:64], pst2[:], AF.Copy)
                ag2in = dram.tile([S_PAD, 128], dt.bfloat16, name=f"ag2in_{k}")
                nc.sync.dma_start(
                    ag2in[0:S, :].rearrange("(c r) e -> r c e", r=128), pm2[:]
                )
                nc.sync.dma_start(ag2in[S:S_PAD, :], zt[:])
                ag2out = dram.tile(
                    [NCORES * S_PAD, 128],
                    dt.bfloat16,
                    name=f"ag2out_{k}",
                    addr_space="Shared",
                )
                nc.gpsimd.collective_compute(
                    "AllGather",
                    mybir.AluOpType.bypass,
                    replica_groups=RG,
                    ins=[ag2in[:].opt()],
                    outs=[ag2out[:].opt()],
                )
                ig = work.tile([128, 1, IG_N], dt.bfloat16, name=f"ig_{k}")
                nc.gpsimd.dma_gather(
                    ig[:], ag2out[:], sb["igidx"][:], IG_N, IG_N, 128, transpose=True
                )
                strip_in = ig[0:64, 0, 0 : STRIP_R * PW].rearrange(
                    "p (r c) -> p r c", c=PW
                )
                if trunc == 4:
                    tr = work.tile([C, S], dt.float32, name="trout")
                    nc.scalar.activation(tr[:], ig[0:64, 0, 0:S], AF.Copy)
                    nc.sync.dma_start(out_p.ap(), tr[:])
                    break

    nc.compile()
    return nc


_CACHE = {}


def kernel(**inputs):
    x = np.asarray(inputs["x"])
    params = {k: np.asarray(v) for k, v in inputs.items() if k not in ("x",)}
    buckets = params.pop("buckets")
    in_maps, slot2pix = _host_prep(x, buckets, params)

    import os
    trunc = int(os.environ.get("KTRUNC", "0"))
    if _CACHE.get("nc") is None:
        _CACHE["nc"] = _build_nc(trunc)
    nc = _CACHE["nc"]

    from concourse.bass_utils import run_bass_kernel_spmd

    trace = bool(int(os.environ.get("KTRACE", "0")))
    res = run_bass_kernel_spmd(
        nc, in_maps, core_ids=list(range(NCORES)), trace=trace
    )
    _CACHE["last_result"] = res
    outs = [res.results[i]["out"] for i in range(NCORES)]
    return _assemble(outs, slot2pix).astype(np.float32)


def kernel_emulate(**inputs):
    """Numpy emulation of the device program (for logic validation)."""
    x = np.asarray(inputs["x"])
    params = {k: np.asarray(v) for k, v in inputs.items() if k not in ("x",)}
    buckets = params.pop("buckets")
    in_maps, slot2pix = _host_prep(x, buckets, params)
    outs = _emulate(in_maps)
    return _assemble(outs, slot2pix).astype(np.float32)



# revision 14
# speedup vs baseline: 1.4042x; 1.4042x over previous
"""Trainium2 Bass kernel for nn_BlockBucket (3x eres_block + basic_block).

Strategy: the per-pixel dynamic conv (filters from a 72-entry embedding table
indexed by `buckets`) is computed as bucket-sorted matmuls: pixels are grouped
by bucket into 64-pixel tiles (host-side index prep only -- all FLOPs on
device), patches are built with one dma_gather per block from an AllGathered
pixel-major conv1 output, and each tile does 9 accumulating K=64 matmuls
against its bucket's filter.  Everything between the 3x3 convs is pointwise and
stays in the bucket-sorted "slot" domain.  2 AllGathers/block (conv1out image,
o_k slots), none after block 3 (host reassembles from per-core slot outputs).
"""

import sys

sys.path.insert(0, "/opt/trn_rl_repo")

import numpy as np
import ml_dtypes

BF16 = ml_dtypes.bfloat16

# problem constants
C = 64
H = W = 64
NPIX = H * W            # 4096
NTYPES = 72
KK = 9                  # 3x3
EMB_DIM = C * (C * KK + 1)
GROUP = 4
NCORES = 8

# layout constants
TS = 64                 # slot tile size
S_MAX = 12              # tiles per core (supports up to 96 tiles globally)
S = S_MAX * TS          # 768 slots per core
S_PAD = S + 64          # +64 guaranteed-zero rows in the AG2 contribution
ZERO_SLOT = S           # rank-0 row index of a zero row (global row = S)
ROWS_PER_CORE = H // NCORES     # 8 image rows per strip
PIX_PER_CORE = ROWS_PER_CORE * W  # 512
PW = 66                 # padded row width for strip layout
STRIP_R = 12            # strip tile rows: 8 interior + 2 halo + 2 margin
IG_N = 896              # image-gather num_idxs (>= STRIP_R*PW=792, %128==0)
PG_N = 3 * S            # patch-gather num_idxs = 2304 (one per dy group, %128==0)
AGROWS = 8 * PW         # padded ag1 rows per core (8 image rows x 66 cols)


def _reflect(v, n=64):
    if v < 0:
        return -v
    if v >= n:
        return 2 * n - 2 - v
    return v


def _wrap_idx(idx, n):
    """int16 index array -> [128, n//16] layout (j -> [j%16, j//16])."""
    assert len(idx) == n and n % 16 == 0
    blk = np.asarray(idx, np.int16).reshape(n // 16, 16).T
    return np.tile(blk, (8, 1))


def _host_prep(x, buckets, params):
    """Build per-core input maps + assembly info. All numpy."""
    x = np.asarray(x, np.float32).reshape(C, NPIX)
    bk = np.asarray(buckets, np.int64).reshape(NPIX)

    # ---- slot assignment (shared by all 3 blocks) ----
    tiles = []  # list of (bucket, [pixels padded with -1 to TS])
    for t in range(NTYPES):
        pix = np.nonzero(bk == t)[0]
        for off in range(0, max(len(pix), 1), TS):
            chunk = pix[off : off + TS]
            if len(chunk) == 0:
                continue
            pad = np.full(TS, -1, np.int64)
            pad[: len(chunk)] = chunk
            tiles.append((t, pad))
    n_tiles = len(tiles)
    assert n_tiles <= NCORES * S_MAX, f"too many tiles: {n_tiles}"
    # round-robin so per-core counts balance
    core_tiles = [[] for _ in range(NCORES)]
    for i, tl in enumerate(tiles):
        core_tiles[i % NCORES].append(tl)
    for i in range(NCORES):
        while len(core_tiles[i]) < S_MAX:
            core_tiles[i].append((0, np.full(TS, -1, np.int64)))

    slot2pix = np.full((NCORES, S), -1, np.int64)     # core, slot -> pixel
    tile_bucket = np.zeros((NCORES, S_MAX), np.int64)
    for i in range(NCORES):
        for t, (b, pads) in enumerate(core_tiles[i]):
            tile_bucket[i, t] = b
            slot2pix[i, t * TS : (t + 1) * TS] = pads
    pix2gslot = np.full(NPIX, -1, np.int64)           # pixel -> global AG2 row
    for i in range(NCORES):
        for s in range(S):
            p = slot2pix[i, s]
            if p >= 0:
                pix2gslot[p] = i * S_PAD + s
    assert (pix2gslot >= 0).all()

    # ---- index arrays ----
    # patch gather: 3 dy-groups, each fetches the horizontal triple
    # (x-1,x,x+1) as one 768B elem from the 66-wide reflect-padded image.
    # idx = reflect(y+dy)*66 + x starts the triple at padded col x.
    pgidx = np.zeros((NCORES, PG_N), np.int64)
    for i in range(NCORES):
        for d in range(3):
            dy = d - 1
            for s in range(S):
                p = slot2pix[i, s]
                if p < 0:
                    v = 0
                else:
                    y, xx = divmod(int(p), W)
                    v = _reflect(y + dy) * PW + xx
                pgidx[i, d * S + s] = v

    igidx = np.full((NCORES, IG_N), ZERO_SLOT, np.int64)  # image gather
    for i in range(NCORES):
        base = 8 * i * PW
        for j in range(STRIP_R * PW):
            r, cc = divmod(base + j, PW)
            y, xx = r - 1, cc - 1
            if 0 <= y < H and 0 <= xx < W:
                igidx[i, j] = pix2gslot[y * W + xx]

    # ---- weights ----
    def embw(emb):
        e = np.asarray(emb, np.float32).reshape(NTYPES, C, C * KK + 1)
        wf = e[:, :, : C * KK].reshape(NTYPES, C, C, KK)  # [t, o, c, kk]
        bias = e[:, :, -1]                                # [t, o]
        return wf, bias

    def conv1_bd(w1):
        # grouped (64,16,3,3) -> block-diag lhsT chunks [ci, kk, o]
        w1 = np.asarray(w1, np.float32)
        out = np.zeros((C, KK, C), np.float32)
        gs = C // GROUP
        for o in range(C):
            g = o // gs
            for cl in range(gs):
                out[g * gs + cl, :, o] = w1[o, cl].reshape(KK)
        return out

    repl = {}
    repl["ident"] = np.eye(128, dtype=np.float32).astype(BF16)
    for k, pre in ((1, "b1"), (2, "b2"), (3, "b3")):
        repl[f"w1bd{k}"] = conv1_bd(params[pre + "_w1"]).astype(BF16)
        repl[f"b1_{k}"] = np.asarray(params[pre + "_b1"], np.float32).reshape(C, 1)
        # w2 (64,64,1,1) -> lhsT [c, o]
        repl[f"w2t{k}"] = (
            np.asarray(params[pre + "_w2"], np.float32).reshape(C, C).T.copy()
        ).astype(BF16)
        repl[f"b2_{k}"] = np.asarray(params[pre + "_b2"], np.float32).reshape(C, 1)
    for k, cn in ((1, "c1"), (2, "c2"), (3, "c3")):
        cw = np.asarray(params[cn + "_w"], np.float32).reshape(C, C * (k + 1))
        # chunks [ci, j, o]
        repl[f"cw{k}"] = (
            cw.reshape(C, k + 1, C).transpose(2, 1, 0).copy()
        ).astype(BF16)
        repl[f"cb{k}"] = np.asarray(params[cn + "_b"], np.float32).reshape(C, 1)

    in_maps = []
    for i in range(NCORES):
        m = dict(repl)
        # x strip [64, 12, 66] zero-padded
        xs = np.zeros((C, STRIP_R, PW), np.float32)
        base = 8 * i * PW
        for j in range(STRIP_R * PW):
            r, cc = divmod(base + j, PW)
            y, xx = r - 1, cc - 1
            if 0 <= y < H and 0 <= xx < W:
                xs[:, j // PW, j % PW] = x[:, y * W + xx]
        m["xstrip"] = xs.astype(BF16)
        m["pgidx"] = _wrap_idx(pgidx[i], PG_N)
        m["igidx"] = _wrap_idx(igidx[i], IG_N)
        x0s = np.zeros((C, S), np.float32)
        sel = slot2pix[i] >= 0
        x0s[:, sel] = x[:, slot2pix[i][sel]]
        m["x0slot"] = x0s.astype(BF16)
        for k in (1, 2, 3):
            emb = params[f"b{k}_emb"]
            wf, bias = embw(emb)
            wloc = np.zeros((C, S_MAX, KK, C), np.float32)  # [ci, t, kk, o]
            bloc = np.zeros((C, S_MAX), np.float32)
            for t in range(S_MAX):
                b = tile_bucket[i, t]
                wloc[:, t, :, :] = wf[b].transpose(1, 2, 0)  # [c, kk, o]
                bloc[:, t] = bias[b]
            m[f"wloc{k}"] = wloc.astype(BF16)
            m[f"bloc{k}"] = bloc
        in_maps.append(m)

    return in_maps, slot2pix


def _emulate_core(m, k_blocks=3):
    """Pure-numpy mirror of the device program for ONE core, given its
    in_map plus the AllGather results (computed by _emulate below)."""
    raise NotImplementedError  # see _emulate


def _emulate(in_maps):
    """Numpy emulation of the full 8-core device program (validates all
    index/layout logic; mirrors device ops incl. bf16 rounding points)."""
    f32 = np.float32

    def bf(a):
        return a.astype(BF16)

    x0s = [np.asarray(in_maps[i]["x0slot"], f32) for i in range(NCORES)]

    strip = [np.asarray(in_maps[i]["xstrip"], f32) for i in range(NCORES)]
    bslots = [[] for _ in range(NCORES)]
    oslot_prev = [None] * NCORES
    out = [None] * NCORES

    for k in (1, 2, 3):
        # conv1 on strips + relu -> padded pixel-major image [4224, 128]
        ag1 = np.zeros((NCORES * AGROWS, 128), f32)
        for i in range(NCORES):
            w1 = np.asarray(in_maps[i][f"w1bd{k}"], f32)
            b1 = in_maps[i][f"b1_{k}"]
            ps = np.zeros((C, 8, 64), f32)
            for j in range(KK):
                dy, dx = j // 3, j % 3
                rhs = strip[i][:, dy : dy + 8, dx : dx + 64]
                ps += np.einsum("co,crw->orw", w1[:, j, :], rhs)
            c1 = np.maximum(ps.reshape(C, PIX_PER_CORE) + b1, 0)
            cp = bf(c1).T.astype(f32)  # [512, C] pixel-major
            for r in range(8):
                for col in range(PW):
                    xx = _reflect(col - 1)
                    ag1[(8 * i + r) * PW + col, :C] = cp[r * W + xx]
        ag1 = bf(ag1)

        ag2 = np.zeros((NCORES * S_PAD, 128), f32)
        for i in range(NCORES):
            m = in_maps[i]
            pidx = m["pgidx"][:16].T.reshape(-1)[:PG_N].astype(np.int64)
            patches = np.zeros((128, KK, S), f32)
            for d in range(3):
                for dd in range(3):
                    patches[:, 3 * d + dd, :] = (
                        np.asarray(ag1, f32)[pidx[d * S : (d + 1) * S] + dd].T
                    )
            wloc = np.asarray(m[f"wloc{k}"], f32)
            bloc = m[f"bloc{k}"]
            lrelu = np.zeros((C, S), f32)
            for t in range(S_MAX):
                acc = np.zeros((C, TS), f32)
                for j in range(KK):
                    rhs = patches[:C, j, t * TS : (t + 1) * TS]
                    acc += wloc[:, t, j, :].T @ rhs
                lrelu[:, t * TS : (t + 1) * TS] = np.maximum(
                    acc + bloc[:, t : t + 1], 0
                )
            lrelu = bf(lrelu).astype(f32)
            xs = x0s[i] if k == 1 else oslot_prev[i]
            w2t = np.asarray(m[f"w2t{k}"], f32)
            ps2 = w2t.T @ lrelu + xs  # identity-matmul residual
            bslot = np.maximum(ps2 + m[f"b2_{k}"], 0)
            bslot = bf(bslot).astype(f32)
            bslots[i].append(bslot)
            chain = [x0s[i]] + bslots[i]
            cwk = np.asarray(m[f"cw{k}"], f32)
            ps3 = np.zeros((C, S), f32)
            for j, rt in enumerate(chain):
                ps3 += cwk[:, j, :].T @ bf(rt.astype(f32)).astype(f32)
            ok = np.maximum(ps3 + m[f"cb{k}"], 0)
            if k == 3:
                out[i] = ok.astype(f32)
            else:
                okb = bf(ok)
                ag2[i * S_PAD : i * S_PAD + S, :C] = okb.T
                oslot_prev[i] = okb.astype(f32)
        if k < 3:
            ag2 = bf(ag2)
            for i in range(NCORES):
                g = _gather_np(ag2, in_maps[i]["igidx"], IG_N)
                strip[i] = (
                    g[:C, : STRIP_R * PW].reshape(C, STRIP_R, PW).astype(f32)
                )
    return out


def _gather_np(src, widx, n):
    """numpy mirror of dma_gather(transpose=True, elem=128): out[128, n]."""
    idx = widx[:16].T.reshape(-1)[:n].astype(np.int64)
    return np.asarray(src, np.float32)[idx].T.copy()  # [128, n]


def _assemble(outs, slot2pix):
    img = np.zeros((C, NPIX), np.float32)
    for i in range(NCORES):
        o = np.asarray(outs[i], np.float32)
        sel = slot2pix[i] >= 0
        img[:, slot2pix[i][sel]] = o[:, np.nonzero(sel)[0]]
    return img.reshape(1, C, H, W)


# ---------------------------------------------------------------------------
# bass graph
# ---------------------------------------------------------------------------


def _build_nc(trunc=0):
    import os
    import concourse.bass as bass
    import concourse.bacc as bacc
    import concourse.mybir as mybir
    import concourse.tile as tile

    dt = mybir.dt
    AF = mybir.ActivationFunctionType
    RG = [list(range(NCORES))]

    nc = bacc.Bacc(
        "TRN2",
        target_bir_lowering=False,
        debug=False,
        num_devices=NCORES,
    )

    # ---- parameters ----
    P = {}

    def param(name, shape, dtype):
        P[name] = nc.declare_dram_parameter(name, list(shape), dtype, False)

    param("xstrip", (C, STRIP_R, PW), dt.bfloat16)
    param("x0slot", (C, S), dt.bfloat16)
    param("ident", (128, 128), dt.bfloat16)
    param("pgidx", (128, PG_N // 16), dt.int16)
    param("igidx", (128, IG_N // 16), dt.int16)
    for k in (1, 2, 3):
        param(f"w1bd{k}", (C, KK, C), dt.bfloat16)
        param(f"b1_{k}", (C, 1), dt.float32)
        param(f"w2t{k}", (C, C), dt.bfloat16)
        param(f"b2_{k}", (C, 1), dt.float32)
        param(f"cw{k}", (C, k + 1, C), dt.bfloat16)
        param(f"cb{k}", (C, 1), dt.float32)
        param(f"wloc{k}", (C, S_MAX, KK, C), dt.bfloat16)
        param(f"bloc{k}", (C, S_MAX), dt.float32)
    out_p = nc.declare_dram_parameter("out", [C, S], dt.float32, True)

    with tile.TileContext(nc) as tc:
        with (
            tc.tile_pool(name="wpool", bufs=1) as wpool,
            tc.tile_pool(name="work", bufs=1) as work,
            tc.tile_pool(name="psA", bufs=2, space=bass.MemorySpace.PSUM) as psA,
            tc.tile_pool(name="psB", bufs=1, space=bass.MemorySpace.PSUM) as psB,
            tc.tile_pool(name="dram", bufs=1, space=bass.MemorySpace.DRAM) as dram,
        ):
            # ---- warm-up collective: absorbs the first-collective barrier
            # (~40us) behind the param loads + block-1 conv work ----
            warm_in = dram.tile([16, 128], dt.bfloat16, name="warm_in")
            warm_out = dram.tile(
                [NCORES * 16, 128], dt.bfloat16, name="warm_out", addr_space="Shared"
            )
            wz = work.tile([16, 128], dt.bfloat16, name="warm_sb")
            nc.vector.memset(wz[:], 0.0)
            nc.sync.dma_start(warm_in[:], wz[:])
            nc.gpsimd.collective_compute(
                "AllGather",
                mybir.AluOpType.bypass,
                replica_groups=RG,
                ins=[warm_in[:].opt()],
                outs=[warm_out[:].opt()],
            )

            # ---- load replicated/static data into SBUF ----
            sb = {}
            for name in P:
                t = wpool.tile(list(P[name].shape), P[name].dtype, name=f"sb_{name}")
                nc.sync.dma_start(t[:], P[name].ap())
                sb[name] = t

            ident = sb["ident"]

            # zero tile for AG2 zero rows
            zt = work.tile([64, 128], dt.bfloat16, name="zt")
            nc.vector.memset(zt[:], 0.0)

            x0s = sb["x0slot"][:, :]

            strip_in = sb["xstrip"][:, :, :]
            bslot = {}
            chain = [x0s]

            for k in (1, 2, 3):
                # ---------- conv1 (strip) ----------
                ps1 = psB.tile([C, 8, 64], dt.float32, name=f"ps1_{k}", tag="ps1")
                for j in range(KK):
                    dy, dx = j // 3, j % 3
                    nc.tensor.matmul(
                        ps1[:],
                        sb[f"w1bd{k}"][:, j, :],
                        strip_in[:, dy : dy + 8, dx : dx + 64],
                        start=(j == 0),
                        stop=(j == KK - 1),
                    )
                c1 = work.tile([C, PIX_PER_CORE], dt.bfloat16, name=f"c1_{k}")
                nc.scalar.activation(
                    c1[:],
                    ps1.rearrange("p a b -> p (a b)"),
                    AF.Relu,
                    bias=sb[f"b1_{k}"][:],
                )

                # ---------- transpose to pixel-major + AG1 (padded strip) ----------
                pm1 = work.tile([128, 4, 128], dt.bfloat16, name=f"pm1_{k}")
                nc.vector.memset(pm1[:], 0.0)
                for cc in range(4):
                    pst = psA.tile([128, 64], dt.bfloat16, name=f"pst1_{k}_{cc}", tag="pst")
                    nc.tensor.transpose(
                        pst[:], c1[:, 128 * cc : 128 * (cc + 1)], ident[0:64, 0:64]
                    )
                    nc.scalar.activation(pm1[:, cc, 0:64], pst[:], AF.Copy)
                # padded strip rows: [cc, parity, 66, 128]; flat row
                # (2*cc+par)*66 + col; col j holds image x=j-1 (reflect at 0/65)
                ag1in = dram.tile([4, 2, PW, 128], dt.bfloat16, name=f"ag1in_{k}")
                for par in range(2):
                    nc.sync.dma_start(
                        ag1in[:, par, 1:65, :].rearrange("c x e -> x c e"),
                        pm1[64 * par : 64 * (par + 1), :, :],
                    )
                    nc.sync.dma_start(
                        ag1in[:, par, 0:1, :].rearrange("c x e -> x c e"),
                        pm1[64 * par + 1 : 64 * par + 2, :, :],
                    )
                    nc.sync.dma_start(
                        ag1in[:, par, 65:66, :].rearrange("c x e -> x c e"),
                        pm1[64 * par + 62 : 64 * par + 63, :, :],
                    )
                c1pm = dram.tile(
                    [NCORES * AGROWS, 128],
                    dt.bfloat16,
                    name=f"c1pm_{k}",
                    addr_space="Shared",
                )
                nc.gpsimd.collective_compute(
                    "AllGather",
                    mybir.AluOpType.bypass,
                    replica_groups=RG,
                    ins=[ag1in[:].opt()],
                    outs=[c1pm[:].opt()],
                )

                # ---------- patch gather: 3 dy-groups of 3-pixel triples ----------
                c1pm_ap = c1pm[:]
                trip_src = bass.AP(
                    c1pm_ap.tensor,
                    c1pm_ap.offset,
                    [[128, NCORES * AGROWS - 2], [1, 384]],
                )
                patches = work.tile([128, KK, S], dt.bfloat16, name=f"patches_{k}")
                for d in range(3):
                    nc.gpsimd.dma_gather(
                        patches[:, 3 * d : 3 * (d + 1), :],
                        trip_src,
                        sb["pgidx"][:, (S // 16) * d : (S // 16) * (d + 1)],
                        S,
                        S,
                        384,
                        elem_step=128,
                        transpose=True,
                    )
                if trunc == 1:
                    tr = work.tile([C, S], dt.float32, name="trout")
                    nc.scalar.activation(tr[:], patches[0:64, 0, 0:S], AF.Copy)
                    nc.sync.dma_start(out_p.ap(), tr[:])
                    break

                # ---------- local conv per tile ----------
                lrelu = work.tile([C, S], dt.bfloat16, name=f"lrelu_{k}")
                for t in range(S_MAX):
                    psl = psA.tile([C, TS], dt.float32, name=f"psl_{k}_{t}", tag="psl")
                    for j in range(KK):
                        nc.tensor.matmul(
                            psl[:],
                            sb[f"wloc{k}"][:, t, j, :],
                            patches[0:64, j, t * TS : (t + 1) * TS],
                            start=(j == 0),
                            stop=(j == KK - 1),
                        )
                    nc.scalar.activation(
                        lrelu[:, t * TS : (t + 1) * TS],
                        psl[:],
                        AF.Relu,
                        bias=sb[f"bloc{k}"][:, t : t + 1],
                    )

                if trunc == 2:
                    tr = work.tile([C, S], dt.float32, name="trout")
                    nc.scalar.activation(tr[:], lrelu[:], AF.Copy)
                    nc.sync.dma_start(out_p.ap(), tr[:])
                    break

                # ---------- conv2 + residual ----------
                xs = chain[-1] if k > 1 else x0s  # o_{k-1} slots (or x0)
                bs = work.tile([C, S], dt.bfloat16, name=f"bslot_{k}")
                for ch0 in range(0, S, 384):
                    sl = slice(ch0, ch0 + 384)
                    ps2 = psA.tile([C, 384], dt.float32, name=f"ps2_{k}_{ch0}", tag="ps2")
                    nc.tensor.matmul(
                        ps2[:], sb[f"w2t{k}"][:], lrelu[:, sl], start=True, stop=False
                    )
                    nc.tensor.matmul(
                        ps2[:], ident[0:64, 0:64], xs[:, sl], start=False, stop=True
                    )
                    nc.scalar.activation(
                        bs[:, sl], ps2[:], AF.Relu, bias=sb[f"b2_{k}"][:]
                    )
                bslot[k] = bs
                chain_k = [x0s] + [bslot[j][:, :] for j in range(1, k + 1)]

                # ---------- basic block ----------
                odt = dt.float32 if k == 3 else dt.bfloat16
                ok = work.tile([C, S], odt, name=f"oslot_{k}")
                for ch0 in range(0, S, 384):
                    sl = slice(ch0, ch0 + 384)
                    ps3 = psA.tile([C, 384], dt.float32, name=f"ps3_{k}_{ch0}", tag="ps2")
                    for j, rt in enumerate(chain_k):
                        nc.tensor.matmul(
                            ps3[:],
                            sb[f"cw{k}"][:, j, :],
                            rt[:, sl],
                            start=(j == 0),
                            stop=(j == len(chain_k) - 1),
                        )
                    nc.scalar.activation(
                        ok[:, sl], ps3[:], AF.Relu, bias=sb[f"cb{k}"][:]
                    )

                if k == 3 or trunc == 3:
                    if trunc == 3 and k != 3:
                        tr = work.tile([C, S], dt.float32, name="trout")
                        nc.scalar.activation(tr[:], ok[:], AF.Copy)
                        nc.sync.dma_start(out_p.ap(), tr[:])
                    else:
                        nc.sync.dma_start(out_p.ap(), ok[:])
                    break

                chain.append(ok[:, :])

                # ---------- AG2 + image gather for next block ----------
                pm2 = work.tile([128, 6, 128], dt.bfloat16, name=f"pm2_{k}")
                nc.vector.memset(pm2[:], 0.0)
                for cc in range(6):
                    pst2 = psA.tile([128, 64], dt.bfloat16, name=f"pst2_{k}_{cc}", tag="pst")
                    nc.tensor.transpose(
                        pst2[:], ok[:, 128 * cc : 128 * (cc + 1)], ident[0:64, 0:64]
                    )
                    nc.scalar.activation(pm2[:, cc, 0:64], pst2[:], AF.Copy)
                ag2in = dram.tile([S_PAD, 128], dt.bfloat16, name=f"ag2in_{k}")
                nc.sync.dma_start(
                    ag2in[0:S, :].rearrange("(c r) e -> r c e", r=128), pm2[:]
                )
                nc.sync.dma_start(ag2in[S:S_PAD, :], zt[:])
                ag2out = dram.tile(
                    [NCORES * S_PAD, 128],
                    dt.bfloat16,
                    name=f"ag2out_{k}",
                    addr_space="Shared",
                )
                nc.gpsimd.collective_compute(
                    "AllGather",
                    mybir.AluOpType.bypass,
                    replica_groups=RG,
                    ins=[ag2in[:].opt()],
                    outs=[ag2out[:].opt()],
                )
                ig = work.tile([128, 1, IG_N], dt.bfloat16, name=f"ig_{k}")
                nc.gpsimd.dma_gather(
                    ig[:], ag2out[:], sb["igidx"][:], IG_N, IG_N, 128, transpose=True
                )
                strip_in = ig[0:64, 0, 0 : STRIP_R * PW].rearrange(
                    "p (r c) -> p r c", c=PW
                )
                if trunc == 4:
                    tr = work.tile([C, S], dt.float32, name="trout")
                    nc.scalar.activation(tr[:], ig[0:64, 0, 0:S], AF.Copy)
                    nc.sync.dma_start(out_p.ap(), tr[:])
                    break

    nc.compile()
    return nc


_CACHE = {}


def kernel(**inputs):
    x = np.asarray(inputs["x"])
    params = {k: np.asarray(v) for k, v in inputs.items() if k not in ("x",)}
    buckets = params.pop("buckets")
    in_maps, slot2pix = _host_prep(x, buckets, params)

    import os
    trunc = int(os.environ.get("KTRUNC", "0"))
    if _CACHE.get("nc") is None:
        _CACHE["nc"] = _build_nc(trunc)
    nc = _CACHE["nc"]

    from concourse.bass_utils import run_bass_kernel_spmd

    trace = bool(int(os.environ.get("KTRACE", "0")))
    res = run_bass_kernel_spmd(
        nc, in_maps, core_ids=list(range(NCORES)), trace=trace
    )
    _CACHE["last_result"] = res
    outs = [res.results[i]["out"] for i in range(NCORES)]
    return _assemble(outs, slot2pix).astype(np.float32)


def kernel_emulate(**inputs):
    """Numpy emulation of the device program (for logic validation)."""
    x = np.asarray(inputs["x"])
    params = {k: np.asarray(v) for k, v in inputs.items() if k not in ("x",)}
    buckets = params.pop("buckets")
    in_maps, slot2pix = _host_prep(x, buckets, params)
    outs = _emulate(in_maps)
    return _assemble(outs, slot2pix).astype(np.float32)

